# revision 7
# baseline (speedup 1.0000x reference)
"""Trilinear feature-grid interpolation (NGLOD single-LOD embedding lookup)
on 8 Trainium2 NeuronCores.

Design:
  * Host builds an expanded table ``table4`` [128^3, 32]: row (x,y,z) holds
    [f(x,y,z), f(x+1,y,z), f(x,y+1,z), f(x+1,y+1,z)].  All 8 trilinear
    corners of a point then live in the 64 consecutive floats (256 B)
    starting at row (x0,y0,z0) — one gather element per point.
  * Points are sharded by x-slab (core = x0//16); each core gets only its
    32 MB slab of the table.  Within a core, points are bucketed by
    (window = local_x//4, z-parity) so each dma_gather instruction reads
    one 8 MB window with 256-byte-aligned rows and int16 window-relative
    indices.
  * Gather uses the Q7 ``dma_gather`` ucode, 8192 indices per instruction,
    rotated over 4 SWDGE queues (all 8 GPSIMD cores generating descriptors
    concurrently, measured ~2.3 ns/index).  The index+gather stage runs
    LOOKAHEAD tiles ahead of the weight/reduce stage so the 4 queues stay
    filled.
  * Per-point normalize/floor/frac/weights and the 8-corner weighted
    reduction run on DVE/ACT, fully overlapped with the gather.
  * The host bucket-sort emits the local cell index per point in the
    "wrapped 16-partition" layout the gather ucode requires; the device
    folds it into window-relative int16 indices.
"""

import numpy as np

RES = 128
F = 8
NCORES = 8
N_PTS = 2_000_000

SLAB_CELLS = 16 * RES * RES          # 262144 cells per core (16 x-planes)
SLAB_ROWS = SLAB_CELLS + 64          # +64 pad rows for odd-parity spill
SHARD = 262_144                      # point slots per core
TILE = 8192                          # points per gather instruction
K = TILE // 128                      # 64  (cols per partition, layout A)
C = TILE // 16                       # 512 (cols per partition, wrapped)
NT = SHARD // TILE                   # 32 tiles
BCAP = 32768                         # bucket capacity (4 tiles)
NBUCKET = 8                          # (window 0..3) x (parity 0..1)
WIN_PAIRS = 32768                    # 256B pair-rows per window
NQ = 4                               # SWDGE queues
LOOKAHEAD = 5                        # gather stage runs this many tiles ahead

_CACHE = {}


# --------------------------------------------------------------------------
# host-side helpers
# --------------------------------------------------------------------------

def host_cells(pts):
    """Exact f32 replication of the reference normalize+floor chain."""
    a = pts.astype(np.float32) + np.float32(1.0)
    b = a * np.float32(0.5)
    xs = b * np.float32(RES - 1)
    x0 = np.clip(np.floor(xs), 0.0, float(RES - 2)).astype(np.int32)
    return xs, x0


def build_table4(features):
    g = np.ascontiguousarray(features, dtype=np.float32).reshape(RES, RES, RES, F)
    ix = np.minimum(np.arange(RES) + 1, RES - 1)
    gx = g[ix]
    t4 = np.concatenate([g, gx, g[:, ix], gx[:, ix]], axis=3)
    return t4.reshape(RES ** 3, 4 * F)


def _layout_perms():
    pos = np.arange(TILE)
    # device DRAM pos p*K+k  <-> slot e = k*128+p
    gatherA = (pos % K) * 128 + pos // K          # pts_a[pos] = slot[gatherA[pos]]
    u = np.arange(128 * C)
    q, c = u // C, u % C
    wsel = c * 16 + (q % 16)                      # cells_w[u] = slot[wsel[u]]
    return gatherA, wsel


def prepare(pts, features):
    """Returns per-core device arrays + bookkeeping to reassemble output."""
    pts = np.asarray(pts, dtype=np.float32)
    n = pts.shape[0]
    xs, x0 = host_cells(pts)
    core = x0[:, 0] >> 4
    lx = x0[:, 0] & 15
    w = lx >> 2
    par = x0[:, 2] & 1
    lcell = (lx.astype(np.int64) * (RES * RES) + x0[:, 1] * RES + x0[:, 2]) \
        .astype(np.float32)

    bucket = core * NBUCKET + w * 2 + par
    order = np.argsort(bucket, kind="stable")
    counts = np.bincount(bucket, minlength=NCORES * NBUCKET)

    pts_slot = np.zeros((NCORES, SHARD, 3), dtype=np.float32)
    cells_slot = np.zeros((NCORES, SHARD), dtype=np.float32)
    for b in range(NBUCKET):
        wb, parb = b // 2, b % 2
        cells_slot[:, b * BCAP:(b + 1) * BCAP] = np.float32(65536 * wb + parb)

    slot_of = np.full(n, -1, dtype=np.int64)
    base = 0
    spill = []
    for cb in range(NCORES * NBUCKET):
        cnt = counts[cb]
        ids = order[base:base + cnt]
        base += cnt
        cc, b = cb // NBUCKET, cb % NBUCKET
        take = min(cnt, BCAP)
        sl = b * BCAP + np.arange(take)
        slot_of[ids[:take]] = cc * SHARD + sl
        pts_slot[cc, sl] = pts[ids[:take]]
        cells_slot[cc, sl] = lcell[ids[:take]]
        if cnt > BCAP:
            spill.append(ids[BCAP:])
    spill = np.concatenate(spill) if spill else np.zeros(0, dtype=np.int64)

    gatherA, wsel = _layout_perms()
    pts_a = pts_slot.reshape(NCORES, NT, TILE, 3)[:, :, gatherA]
    pts_a = np.ascontiguousarray(pts_a.reshape(NCORES, SHARD, 3))
    cells_w = cells_slot.reshape(NCORES, NT, TILE)[:, :, wsel]
    cells_w = np.ascontiguousarray(cells_w.reshape(NCORES, NT * 128 * C))

    t4 = build_table4(features)
    t4p = np.vstack([t4, np.zeros((64, 4 * F), np.float32)])
    slabs = np.stack([
        t4p[cc * SLAB_CELLS: cc * SLAB_CELLS + SLAB_ROWS] for cc in range(NCORES)
    ])

    return pts_a, cells_w, slabs, slot_of, spill, gatherA


def assemble_output(dev_outs, slot_of, spill, pts, features, gatherA, n):
    out_slot = np.stack(dev_outs).reshape(NCORES, NT, TILE, F)
    inv = np.empty(TILE, dtype=np.int64)
    inv[gatherA] = np.arange(TILE)
    out_slot = out_slot[:, :, inv].reshape(NCORES * SHARD, F)
    res = out_slot[slot_of]
    if len(spill):
        res[spill] = _host_trilinear(pts[spill], features)
    return res[:n] if res.shape[0] >= n else res


def _host_trilinear(pts, features):
    res = RES
    xs, x0 = host_cells(pts)
    f = xs - x0.astype(np.float32)
    ix, iy, iz = x0[:, 0], x0[:, 1], x0[:, 2]
    fx, fy, fz = f[:, 0], f[:, 1], f[:, 2]
    out = np.zeros((pts.shape[0], features.shape[1]), dtype=np.float32)
    for dx in (0, 1):
        wx = fx if dx else (1.0 - fx)
        for dy in (0, 1):
            wy = fy if dy else (1.0 - fy)
            for dz in (0, 1):
                wz = fz if dz else (1.0 - fz)
                idx = (ix + dx) * res * res + (iy + dy) * res + (iz + dz)
                out += features[idx] * (wx * wy * wz)[:, None]
    return out


# --------------------------------------------------------------------------
# device kernel
# --------------------------------------------------------------------------

def build_nc(ntiles=NT, num_devices=NCORES, reps=1):
    import concourse.bacc as bacc
    import concourse.tile as tile
    from concourse import mybir

    f32 = mybir.dt.float32
    i32 = mybir.dt.int32
    i16 = mybir.dt.int16
    AF = mybir.ActivationFunctionType
    ALU = mybir.AluOpType
    AX = mybir.AxisListType

    nc = bacc.Bacc("TRN2", debug=False, enable_asserts=False,
                   num_devices=num_devices, num_swdge_queues=NQ,
                   detect_race_conditions=False)
    shard = ntiles * TILE
    pts_a = nc.dram_tensor("pts_a", [shard, 3], f32, kind="ExternalInput").ap()
    cells_w = nc.dram_tensor("cells_w", [ntiles * 128 * C], f32,
                             kind="ExternalInput").ap()
    slab = nc.dram_tensor("slab", [SLAB_ROWS, 4 * F], f32,
                          kind="ExternalInput").ap()
    out = nc.dram_tensor("out", [shard, F], f32, kind="ExternalOutput").ap()

    pts_a_t = pts_a.rearrange("(t p k) c -> t p k c", p=128, k=K)
    cells_t = cells_w.rearrange("(t q c) -> t q c", q=128, c=C)
    out_t = out.rearrange("(t p k) f -> t p k f", p=128, k=K)
    slab_flat = slab.rearrange("v f -> (v f)")

    win_aps = {}
    for wv in range(4):
        for pv in range(2):
            start = wv * (WIN_PAIRS * 64) + pv * 32
            win_aps[(wv, pv)] = slab_flat[start:start + WIN_PAIRS * 64] \
                .rearrange("(v f) -> v f", f=64)

    L = LOOKAHEAD

    with tile.TileContext(nc) as tc:
        with tc.tile_pool(name="io", bufs=3) as iop, \
             tc.tile_pool(name="cwp", bufs=L + 3) as cwp, \
             tc.tile_pool(name="gp", bufs=L + 1) as gp, \
             tc.tile_pool(name="ip", bufs=L + 3) as ipool, \
             tc.tile_pool(name="tp", bufs=2) as tp:
            gtiles = {}
            for rep in range(reps):
                for u in range(ntiles + L):
                    if u < ntiles:
                        t = u
                        b = t // 4
                        wv, pv = b // 2, b % 2
                        # ---- index + gather stage (runs ahead) ----
                        cw = cwp.tile([128, C], f32, tag="cw")
                        nc.sync.dma_start(out=cw[:], in_=cells_t[t])
                        t1 = ipool.tile([128, C], f32, tag="t1")
                        nc.vector.tensor_scalar(
                            out=t1[:], in0=cw[:],
                            scalar1=0.5, scalar2=-(wv * 32768.0 + pv * 0.5),
                            op0=ALU.mult, op1=ALU.add)
                        idx = ipool.tile([128, C], i16, tag="idx")
                        nc.vector.tensor_copy(out=idx[:], in_=t1[:])
                        g = gp.tile([128, K, 64], f32, tag="g")
                        nc.gpsimd.dma_gather(
                            g[:], win_aps[(wv, pv)], idx[:], TILE, TILE, 64,
                            queue_num=t % NQ, single_packet=False)
                        gtiles[t] = g

                    if u >= L:
                        t = u - L
                        g = gtiles.pop(t)
                        # ---- weights path (layout A) ----
                        p3 = iop.tile([128, K, 3], f32, tag="p3")
                        nc.sync.dma_start(out=p3[:], in_=pts_a_t[t])
                        xs = tp.tile([128, K, 3], f32, tag="xs")
                        nc.vector.tensor_scalar(out=xs[:], in0=p3[:],
                                                scalar1=1.0, scalar2=0.5,
                                                op0=ALU.add, op1=ALU.mult)
                        nc.vector.tensor_scalar_mul(xs[:], xs[:], float(RES - 1))
                        x0i = tp.tile([128, K, 3], i32, tag="x0i")
                        nc.vector.tensor_copy(out=x0i[:], in_=xs[:])
                        x0f = tp.tile([128, K, 3], f32, tag="x0f")
                        nc.vector.tensor_copy(out=x0f[:], in_=x0i[:])
                        fr = tp.tile([128, K, 3], f32, tag="fr")
                        nc.vector.tensor_sub(fr[:], xs[:], x0f[:])
                        sg = tp.tile([128, K, 3], f32, tag="sg")
                        nc.scalar.activation(out=sg[:], in_=fr[:], func=AF.Sign)
                        nc.vector.tensor_scalar_min(sg[:], sg[:], 0.0)
                        nc.vector.tensor_add(x0f[:], x0f[:], sg[:])
                        nc.vector.tensor_scalar_min(x0f[:], x0f[:],
                                                    float(RES - 2))
                        nc.vector.tensor_sub(fr[:], xs[:], x0f[:])

                        wx = tp.tile([128, K, 2], f32, tag="wx")
                        wy = tp.tile([128, K, 2], f32, tag="wy")
                        wz = tp.tile([128, K, 2], f32, tag="wz")
                        for wt, ch in ((wx, 0), (wy, 1), (wz, 2)):
                            nc.vector.tensor_scalar(
                                out=wt[:, :, 0], in0=fr[:, :, ch],
                                scalar1=-1.0, scalar2=1.0,
                                op0=ALU.mult, op1=ALU.add)
                            nc.vector.tensor_copy(out=wt[:, :, 1],
                                                  in_=fr[:, :, ch])
                        wxy = tp.tile([128, K, 2, 2], f32, tag="wxy")
                        nc.vector.tensor_tensor(
                            out=wxy[:],
                            in0=wy[:].unsqueeze(3).broadcast_to([128, K, 2, 2]),
                            in1=wx[:].unsqueeze(2).broadcast_to([128, K, 2, 2]),
                            op=ALU.mult)
                        w8 = tp.tile([128, K, 2, 4], f32, tag="w8")
                        nc.vector.tensor_tensor(
                            out=w8[:],
                            in0=wz[:].unsqueeze(3).broadcast_to([128, K, 2, 4]),
                            in1=wxy[:].rearrange("p k a b -> p k (a b)")
                                      .unsqueeze(2).broadcast_to([128, K, 2, 4]),
                            op=ALU.mult)

                        # ---- combine ----
                        g4 = g[:].rearrange("p k (a b) -> p k a b", a=8)
                        nc.vector.tensor_tensor(
                            out=g4,
                            in0=g4,
                            in1=w8[:].rearrange("p k a b -> p k (a b)")
                                     .unsqueeze(3).broadcast_to([128, K, 8, 8]),
                            op=ALU.mult)
                        o = iop.tile([128, K, F], f32, tag="o")
                        nc.vector.tensor_reduce(
                            out=o[:],
                            in_=g4.transpose([0, 1, 3, 2]),
                            axis=AX.X,
                            op=ALU.add)
                        nc.sync.dma_start(out=out_t[t], in_=o[:])

    nc.compile()
    return nc


# --------------------------------------------------------------------------
# entry point
# --------------------------------------------------------------------------

def kernel(pts, features):
    pts = np.asarray(pts, dtype=np.float32)
    features = np.asarray(features, dtype=np.float32)
    n = pts.shape[0]

    pts_a, cells_w, slabs, slot_of, spill, gatherA = prepare(pts, features)

    if "nc" not in _CACHE:
        _CACHE["nc"] = build_nc()
    nc = _CACHE["nc"]

    from concourse.bass_utils import run_bass_kernel_spmd
    in_maps = [
        {"pts_a": pts_a[i], "cells_w": cells_w[i], "slab": slabs[i]}
        for i in range(NCORES)
    ]
    res = run_bass_kernel_spmd(nc, in_maps, core_ids=list(range(NCORES)))
    dev_outs = [res.results[i]["out"] for i in range(NCORES)]
    return assemble_output(dev_outs, slot_of, spill, pts, features, gatherA, n)
